# revision 1
# baseline (speedup 1.0000x reference)
"""Trainium2 Bass kernel for nn_ConvEmbeddingXY (retrieval_knn).

Problem: B=32 batches of N=1000 2-D points. Per point: node embedding
(x @ W1 + b1), 10-NN by squared distance (incl. self), neighbor coords
sorted by x and by y feed two tiny convs, conv outputs go through W2 and
sum with the node embedding.

Strategy (data-parallel over B across 8 cores, 4 batches/core):
  - distances via PE matmul on centered coords: u = 2*xc_i.xc_j - r_j - r_i
    (= -d2 up to rounding; centering keeps cancellation error ~1e-7)
  - exact top-10 via DVE max8/max_index/match_replace (duplicate-aware,
    ascending-index ties = jax top_k tie-break), 16 candidates kept
  - candidate (x,y) pairs fetched with GPSIMD ap_gather (core-shared index
    stream == the natural [row, cand] uint16 layout), then a masked
    reduction picks each row's own 16 pairs out of the shared stream
  - refine: d2 recomputed exactly like the reference ((x_i-x_j)^2+(y_i-y_j)^2
    in f32) on the 16 candidates; top-10 marked via match_replace sentinel
  - per-axis sort of the 10 pairs via max8 on negated coords (values are the
    sorted coords; companions via a width-16 one-hot multiply+reduce)
  - all four contractions (node emb, conv_x, conv_y, W2, biases) are folded
    on the host into one [43, H] matrix; per chunk the 43-feature vectors are
    PE-transposed and one matmul produces the [128, H] output tile.

The whole computation is one Bass/Tile program; only input formatting
(centering, transposes, weight folding) happens on the host.
"""

import numpy as np

B, N, K, H, C = 32, 1000, 10, 128, 2
NPAD = 1024
NCORES = 8
BL = B // NCORES          # batches per core
NCHUNK = NPAD // 128      # 128-point chunks per batch
NF = 2 + 2 * K + 2 * K + 1  # 43 features: x,y | sorted_x pairs | sorted_y pairs | 1

_SENT = -1.0e30


def _split_multiwaits(nc, mybir):
    """This container's walrus build accepts at most ONE sync-wait command per
    instruction. Tile attaches several; redistribute extras onto same-engine
    NoOp carriers placed immediately before the instruction."""
    counter = 0
    for fn in nc.m.functions:
        for blk in fn.blocks:
            insts = blk.instructions
            new = []
            changed = False
            for inst in insts:
                si = inst.sync_info
                waits = list(si.on_wait) if (si is not None and si.on_wait) else []
                if len(waits) > 1:
                    for w in waits[:-1]:
                        counter += 1
                        nop = mybir.InstNoOp(
                            name=f"I-waitcarrier-{counter}", ins=[], outs=[]
                        )
                        nop.engine = inst.engine
                        nop.sync_info = mybir.SyncInfo(on_wait=[w], on_update=[])
                        new.append(nop)
                    inst.sync_info = mybir.SyncInfo(
                        on_wait=[waits[-1]],
                        on_update=list(si.on_update) if si.on_update else [],
                    )
                    changed = True
                new.append(inst)
            if changed:
                blk.instructions = new


def _build_program(debug=False, split=True):
    import concourse.bass as bass
    import concourse.mybir as mybir
    from concourse.tile import TileContext

    f32 = mybir.dt.float32
    u16 = mybir.dt.uint16
    u8 = mybir.dt.uint8
    i16 = mybir.dt.int16
    AF = mybir.ActivationFunctionType
    OP = mybir.AluOpType

    nc = bass.Bass()

    lhsrc = nc.dram_tensor("lhsrc", [BL, 3, NPAD], f32, kind="ExternalInput")
    rhsrc = nc.dram_tensor("rhsrc", [BL, 3, NPAD], f32, kind="ExternalInput")
    negrr = nc.dram_tensor("negrr", [BL, NPAD], f32, kind="ExternalInput")
    xyraw = nc.dram_tensor("xyraw", [BL, NPAD, 2], f32, kind="ExternalInput")
    negxy = nc.dram_tensor("negxy", [BL, NPAD, 2], f32, kind="ExternalInput")
    pmask_d = nc.dram_tensor("pmask", [128, 16], f32, kind="ExternalInput")
    iota16_d = nc.dram_tensor("iota16", [128, 16], f32, kind="ExternalInput")
    ident_d = nc.dram_tensor("ident", [128, 128], f32, kind="ExternalInput")
    mt_d = nc.dram_tensor("mt", [NF, 128], f32, kind="ExternalInput")

    y = nc.dram_tensor("y", [BL, N, H], f32, kind="ExternalOutput")
    if debug:
        d_idx = nc.dram_tensor("d_idx", [BL, NCHUNK, 128, 16], u16, kind="ExternalOutput")
        d_cxy = nc.dram_tensor("d_cxy", [BL, NCHUNK, 128, 32], f32, kind="ExternalOutput")
        d_mask = nc.dram_tensor("d_mask", [BL, NCHUNK, 128, 16], f32, kind="ExternalOutput")
        d_f = nc.dram_tensor("d_f", [BL, NCHUNK, 128, NF], f32, kind="ExternalOutput")

    with TileContext(nc) as tc:
        with (
            tc.tile_pool(name="const", bufs=1) as constp,
            tc.tile_pool(name="batch", bufs=2) as batchp,
            tc.tile_pool(name="big", bufs=3) as bigp,
            tc.tile_pool(name="small", bufs=4) as smallp,
            tc.tile_pool(name="psumG", bufs=2, space="PSUM") as psumGp,
            tc.tile_pool(name="psumT", bufs=2, space="PSUM") as psumTp,
            tc.tile_pool(name="psumO", bufs=2, space="PSUM") as psumOp,
        ):
            pmask = constp.tile([128, 16], f32)
            iota16 = constp.tile([128, 16], f32)
            ident = constp.tile([128, 128], f32)
            mt = constp.tile([NF, 128], f32)
            ones1 = constp.tile([1, 128], f32)
            nc.sync.dma_start(pmask[:], pmask_d[:])
            nc.sync.dma_start(iota16[:], iota16_d[:])
            nc.sync.dma_start(ident[:], ident_d[:])
            nc.sync.dma_start(mt[:], mt_d[:])
            nc.vector.memset(ones1[:], 1.0)

            for b in range(BL):
                lhs_sb = batchp.tile([3, NPAD], f32, tag="lhs")
                rhs_sb = batchp.tile([3, NPAD], f32, tag="rhs")
                xytab = batchp.tile([128, 2 * NPAD], f32, tag="xytab")
                nc.sync.dma_start(lhs_sb[:], lhsrc[b])
                nc.sync.dma_start(rhs_sb[:], rhsrc[b])
                xy0 = batchp.tile([1, 2 * NPAD], f32, tag="xy0")
                nc.sync.dma_start(xy0[:], xyraw[b].rearrange("n c -> (n c)").unsqueeze(0))
                # broadcast the coord table to all partitions via a ones-matmul
                for c0 in range(0, 2 * NPAD, 512):
                    psum_bc = psumOp.tile([128, 512], f32, tag="o")
                    nc.tensor.matmul(psum_bc[:], ones1[:], xy0[:, c0 : c0 + 512],
                                     start=True, stop=True)
                    nc.scalar.activation(xytab[:, c0 : c0 + 512], psum_bc[:],
                                         AF.Copy, bias=0.0, scale=1.0)
                xytab3 = xytab[:].rearrange("p (n c) -> p n c", n=NPAD, c=2)

                for t in range(NCHUNK):
                    n0 = 128 * t
                    rows = min(128, N - n0)
                    if rows <= 0:
                        break

                    # --- distances: u = 2 xc_i . xc_j - r_j - r_i  (= -d2) ---
                    psum_g = psumGp.tile([128, NPAD], f32, tag="g")
                    lhsT = lhs_sb[:, n0 : n0 + 128]
                    nc.tensor.matmul(psum_g[:, 0:512], lhsT, rhs_sb[:, 0:512],
                                     start=True, stop=True)
                    nc.tensor.matmul(psum_g[:, 512:1024], lhsT, rhs_sb[:, 512:1024],
                                     start=True, stop=True)

                    negr_c = smallp.tile([128, 1], f32, tag="negr")
                    nc.sync.dma_start(negr_c[:], negrr[b, n0 : n0 + 128].unsqueeze(1))

                    u = bigp.tile([128, NPAD], f32, tag="u")
                    nc.scalar.activation(u[:, 0:512], psum_g[:, 0:512], AF.Identity,
                                         bias=negr_c[:], scale=1.0)
                    nc.scalar.activation(u[:, 512:1024], psum_g[:, 512:1024], AF.Identity,
                                         bias=negr_c[:], scale=1.0)

                    # --- coarse top-16 (exact up to ~1e-7 matmul rounding) ---
                    m8 = smallp.tile([128, 8], f32, tag="m8")
                    idx16 = smallp.tile([128, 16], u16, tag="idx16")
                    nc.vector.max(out=m8[:], in_=u[:])
                    nc.vector.max_index(out=idx16[:, 0:8], in_max=m8[:], in_values=u[:])
                    nc.vector.match_replace(out=u[:], in_to_replace=m8[:],
                                            in_values=u[:], imm_value=_SENT)
                    m8b = smallp.tile([128, 8], f32, tag="m8b")
                    nc.vector.max(out=m8b[:], in_=u[:])
                    nc.vector.max_index(out=idx16[:, 8:16], in_max=m8b[:], in_values=u[:])

                    # --- gather candidate (x,y) pairs (core-shared stream) ---
                    # indirect_copy with d=2 consumes idx values in flat-element
                    # units and fetches d consecutive elements -> double indices
                    idx2 = smallp.tile([128, 16], u16, tag="idx2")
                    nc.vector.tensor_scalar(out=idx2[:], in0=idx16[:], scalar1=2,
                                            scalar2=None, op0=OP.mult)
                    gath = bigp.tile([128, 512], f32, tag="gath")
                    nc.gpsimd.indirect_copy(
                        out=gath[:].rearrange("p (i c) -> p i c", i=256, c=2),
                        data=xytab3,
                        idxs=idx2[:],
                        i_know_ap_gather_is_preferred=True,
                    )
                    # out[p, m*16+s, c] = xy[cand m of row 16k+s]; keep s == p%16
                    tmp = bigp.tile([128, 512], f32, tag="gtmp")
                    gv = gath[:].rearrange("p (m s c) -> p m s c", m=16, s=16, c=2)
                    pm = pmask[:].unsqueeze(1).unsqueeze(3).to_broadcast([128, 16, 16, 2])
                    nc.vector.tensor_tensor(
                        out=tmp[:].rearrange("p (m s c) -> p m s c", m=16, s=16, c=2),
                        in0=gv, in1=pm, op=OP.mult)
                    cxy16 = smallp.tile([128, 32], f32, tag="cxy16")
                    nc.vector.tensor_reduce(
                        out=cxy16[:],
                        in_=tmp[:].rearrange("p (m s c) -> p m s c", m=16, s=16, c=2)
                            .transpose([0, 1, 3, 2]),
                        axis=mybir.AxisListType.X, op=OP.add)
                    cx16 = cxy16[:, 0:32:2]
                    cy16 = cxy16[:, 1:32:2]

                    # --- exact refine: d2 recomputed as in the reference ---
                    negxy_c = smallp.tile([128, 2], f32, tag="negxyc")
                    nc.sync.dma_start(negxy_c[:], negxy[b, n0 : n0 + 128])
                    dx2 = smallp.tile([128, 16], f32, tag="dx2")
                    dy2 = smallp.tile([128, 16], f32, tag="dy2")
                    nc.scalar.activation(dx2[:], cx16, AF.Square,
                                         bias=negxy_c[:, 0:1], scale=1.0)
                    nc.scalar.activation(dy2[:], cy16, AF.Square,
                                         bias=negxy_c[:, 1:2], scale=1.0)
                    s16 = smallp.tile([128, 16], f32, tag="s16")
                    # s16 = -dx2 - dy2 = -(d2) exactly
                    nc.vector.scalar_tensor_tensor(out=s16[:], in0=dx2[:], scalar=-1.0,
                                                   in1=dy2[:], op0=OP.mult,
                                                   op1=OP.subtract)
                    mc1 = smallp.tile([128, 8], f32, tag="mc1")
                    nc.vector.max(out=mc1[:], in_=s16[:])
                    nc.vector.match_replace(out=s16[:], in_to_replace=mc1[:],
                                            in_values=s16[:], imm_value=_SENT)
                    mc2 = smallp.tile([128, 8], f32, tag="mc2")
                    nc.vector.max(out=mc2[:], in_=s16[:])
                    mrb = smallp.tile([128, 8], f32, tag="mrb")
                    nc.vector.memset(mrb[:], _SENT)
                    nc.vector.tensor_copy(out=mrb[:, 0:2], in_=mc2[:, 0:2])
                    nc.vector.match_replace(out=s16[:], in_to_replace=mrb[:],
                                            in_values=s16[:], imm_value=_SENT)
                    mask16 = smallp.tile([128, 16], u8, tag="mask16")
                    nc.vector.tensor_scalar(out=mask16[:], in0=s16[:], scalar1=_SENT,
                                            scalar2=None, op0=OP.is_equal)

                    # --- F assembly ---
                    F = smallp.tile([128, NF], f32, tag="F")
                    nc.sync.dma_start(F[:, 0:2], xyraw[b, n0 : n0 + 128])
                    nc.vector.memset(F[:, 42:43], 1.0)

                    # one sort pass per axis
                    for axis, (key_c, comp_c, col0) in enumerate(
                        [(cx16, cy16, 2), (cy16, cx16, 22)]
                    ):
                        # key = mask ? -coord : -4  (max8-desc == coord asc)
                        negk = smallp.tile([128, 16], f32, tag=f"negk{axis}")
                        nc.scalar.activation(negk[:], key_c, AF.Identity,
                                             bias=0.0, scale=-1.0)
                        kx = smallp.tile([128, 16], f32, tag=f"kx{axis}")
                        nc.vector.memset(kx[:], -4.0)
                        nc.vector.copy_predicated(out=kx[:], mask=mask16[:], data=negk[:])
                        mk1 = smallp.tile([128, 8], f32, tag=f"mk1{axis}")
                        ordx = smallp.tile([128, 16], u16, tag=f"ord{axis}")
                        nc.vector.max(out=mk1[:], in_=kx[:])
                        nc.vector.max_index(out=ordx[:, 0:8], in_max=mk1[:], in_values=kx[:])
                        nc.vector.match_replace(out=kx[:], in_to_replace=mk1[:],
                                                in_values=kx[:], imm_value=-4.0)
                        mk2 = smallp.tile([128, 8], f32, tag=f"mk2{axis}")
                        nc.vector.max(out=mk2[:], in_=kx[:])
                        nc.vector.max_index(out=ordx[:, 8:16], in_max=mk2[:], in_values=kx[:])
                        # sorted key coords = -(mk values); write strided into F
                        # axis 0 (sort by x): x at col0+2r, y at col0+1+2r
                        # axis 1 (sort by y): y values at col0+1+2r, x comp at col0+2r
                        vcol = col0 if axis == 0 else col0 + 1
                        ccol = col0 + 1 if axis == 0 else col0
                        nc.scalar.activation(
                            F[:, vcol : vcol + 16 : 2], mk1[:], AF.Identity,
                            bias=0.0, scale=-1.0)
                        nc.scalar.activation(
                            F[:, vcol + 16 : vcol + 20 : 2], mk2[:, 0:2], AF.Identity,
                            bias=0.0, scale=-1.0)
                        # companion via one-hot over the 16 candidate slots
                        ordf = smallp.tile([128, 10], f32, tag=f"ordf{axis}")
                        nc.vector.tensor_copy(out=ordf[:], in_=ordx[:, 0:10])
                        oh = smallp.tile([128, 160], f32, tag=f"oh{axis}")
                        oh3 = oh[:].rearrange("p (r j) -> p r j", r=10, j=16)
                        nc.vector.tensor_tensor(
                            out=oh3,
                            in0=ordf[:].unsqueeze(2).to_broadcast([128, 10, 16]),
                            in1=iota16[:].unsqueeze(1).to_broadcast([128, 10, 16]),
                            op=OP.is_equal)
                        ohm = smallp.tile([128, 160], f32, tag=f"ohm{axis}")
                        nc.vector.tensor_tensor(
                            out=ohm[:].rearrange("p (r j) -> p r j", r=10, j=16),
                            in0=oh3,
                            in1=comp_c.unsqueeze(1).to_broadcast([128, 10, 16]),
                            op=OP.mult)
                        nc.vector.tensor_reduce(
                            out=F[:, ccol : ccol + 20 : 2],
                            in_=ohm[:].rearrange("p (r j) -> p r j", r=10, j=16),
                            axis=mybir.AxisListType.X, op=OP.add)

                    # --- output: out = F @ MT via PE transpose + matmul ---
                    psum_t = psumTp.tile([NF, 128], f32, tag="ft")
                    nc.tensor.transpose(psum_t[:], F[:], ident[:])
                    ft_sb = smallp.tile([NF, 128], f32, tag="ftsb")
                    nc.scalar.activation(ft_sb[:], psum_t[:], AF.Copy, bias=0.0, scale=1.0)
                    psum_o = psumOp.tile([128, 128], f32, tag="o")
                    nc.tensor.matmul(psum_o[:], ft_sb[:], mt[:], start=True, stop=True)
                    out_sb = smallp.tile([128, 128], f32, tag="outsb")
                    nc.scalar.activation(out_sb[:], psum_o[:], AF.Copy, bias=0.0, scale=1.0)
                    nc.sync.dma_start(y[b, n0 : n0 + rows, :], out_sb[0:rows, :])

                    if debug:
                        nc.sync.dma_start(d_idx[b, t], idx16[:])
                        nc.sync.dma_start(d_cxy[b, t], cxy16[:])
                        nc.sync.dma_start(d_mask[b, t], mask16[:])
                        nc.sync.dma_start(d_f[b, t], F[:])

    if split:
        _split_multiwaits(nc, mybir)
    return nc


def _host_prep(x, Wx, bx, Wy, by, W1, b1, W2, b2):
    """Build per-core input maps."""
    x = np.asarray(x, dtype=np.float32)
    xc = (x.astype(np.float64) - 0.5).astype(np.float32)  # centered, for distances
    r = (xc[..., 0] * xc[..., 0] + xc[..., 1] * xc[..., 1]).astype(np.float32)

    lhsrc = np.zeros((B, 3, NPAD), np.float32)
    lhsrc[:, 0, :N] = 2.0 * xc[..., 0]
    lhsrc[:, 1, :N] = 2.0 * xc[..., 1]
    lhsrc[:, 2, :N] = -1.0
    rhsrc = np.zeros((B, 3, NPAD), np.float32)
    rhsrc[:, 0, :N] = xc[..., 0]
    rhsrc[:, 1, :N] = xc[..., 1]
    rhsrc[:, 2, :N] = r
    rhsrc[:, 2, N:] = 1.0e30
    negrr = np.zeros((B, NPAD), np.float32)
    negrr[:, :N] = -r
    xyraw = np.zeros((B, NPAD, 2), np.float32)
    xyraw[:, :N] = x
    negxy = np.zeros((B, NPAD, 2), np.float32)
    negxy[:, :N] = -x

    pmask = np.zeros((128, 16), np.float32)
    pmask[np.arange(128), np.arange(128) % 16] = 1.0
    iota16 = np.tile(np.arange(16, dtype=np.float32), (128, 1))
    ident = np.eye(128, dtype=np.float32)

    # fold all contractions into MT [43, H]
    W1_, W2_ = np.asarray(W1, np.float64), np.asarray(W2, np.float64)
    Wx_, Wy_ = np.asarray(Wx, np.float64), np.asarray(Wy, np.float64)
    bx_, by_ = np.asarray(bx, np.float64), np.asarray(by, np.float64)
    b1_, b2_ = np.asarray(b1, np.float64), np.asarray(b2, np.float64)
    mt = np.zeros((NF, H), np.float64)
    mt[0:2, :] = W1_                       # node embedding
    for k in range(K):
        for c in range(C):
            mt[2 + 2 * k + c, :] = Wx_[:, c, k] @ W2_      # sorted_x conv
            mt[22 + 2 * k + c, :] = Wy_[:, c, k] @ W2_     # sorted_y conv
    mt[42, :] = b1_ + b2_ + (bx_ + by_) @ W2_
    mt = mt.astype(np.float32)

    in_maps = []
    for core in range(NCORES):
        sl = slice(core * BL, (core + 1) * BL)
        in_maps.append({
            "lhsrc": lhsrc[sl], "rhsrc": rhsrc[sl], "negrr": negrr[sl],
            "xyraw": xyraw[sl], "negxy": negxy[sl],
            "pmask": pmask, "iota16": iota16, "ident": ident, "mt": mt,
        })
    return in_maps


_CACHE = {}


def _get_program(debug=False):
    key = bool(debug)
    if key not in _CACHE:
        _CACHE[key] = _build_program(debug=debug)
    return _CACHE[key]


def kernel(x, Wx, bx, Wy, by, W1, b1, W2, b2, _debug=False, _trace=False):
    from concourse.bass_utils import run_bass_kernel_spmd

    nc = _get_program(debug=_debug)
    in_maps = _host_prep(x, Wx, bx, Wy, by, W1, b1, W2, b2)
    res = run_bass_kernel_spmd(nc, in_maps, list(range(NCORES)), trace=_trace)
    out = np.concatenate([res.results[i]["y"] for i in range(NCORES)], axis=0)
    if _debug or _trace:
        kernel._last = res
    return out

